# revision 6
# baseline (speedup 1.0000x reference)
"""Cosine-similarity attention kernel for Trainium2 (8 NeuronCores, SPMD).

Problem: query [16,16,1,128], key [16,16,4096,128], mask [16,4096] int32
  scores[b,h,l] = <q,k_l> / max(|q||k_l|, 1e-8);  masked softmax over l.
Output: p_attn [16,16,4096] float32.

Sharding: batch dim split across 8 cores (2 batches/core, 32 (b,h) rows).

Per-core dataflow (l = j*512 + g*128 + p):
  - K slabs DMA'd per (j, bh) as natural tiles [128(p), 4(g), 128(d)] fp32r
  - PE transposes (fp32r, 2x faster than fp32) -> PSUM K^T [128(d), 512(l)]
  - DVE copy-drain  -> KT  (fp32r)    [dots moving operand]
  - ACT square-drain -> K2T (fp32r)   [norms moving operand]
  - dots:  accumulate 32 masked-Q matmuls  (col bh = q_bh, rest 0) -> psum [32, 512]
  - norms: accumulate 32 masked-1s matmuls (col bh = ones)         -> psum [32, 512]
    Both land directly in [bh, l] layout.
  - per-j epilogue: rk = exp(-0.5*ln(qn2*kn2)); e = exp(dots*rk)*mask; partial sums
  - tail: p = e / sum(e);  one 512KB store per core.

softmax max-subtraction is dropped: scores are cosine similarities in [-1,1],
masked entries are multiplied by 0 after exp (identical to exp(-1e9) -> 0).
"""

import sys

if "/opt/trn_rl_repo" not in sys.path:
    sys.path.insert(0, "/opt/trn_rl_repo")

import numpy as np

import concourse.bacc as bacc
import concourse.tile as tile
from concourse import mybir
from concourse.bass_utils import run_bass_kernel_spmd
from concourse.masks import make_identity

F32 = mybir.dt.float32
F32R = mybir.dt.float32r
I32 = mybir.dt.int32
AF = mybir.ActivationFunctionType
AX = mybir.AxisListType

B, H, L, D = 16, 16, 4096, 128
NCORES = 8
BLOC = B // NCORES  # batches per core
NBH = BLOC * H  # 32 (b,h) rows per core
LB = 512  # lambda block size
NG = LB // 128  # tiles per block

_ONE_SET = "natural_log_exp_and_others"  # contains Copy/Identity/Square/Ln/Exp


class _Bacc(bacc.Bacc):
    """Bacc that pins all activations to a single ACT table set, avoiding
    ~2.7us table reloads when Square and Ln/Exp interleave."""

    PIN_TABLES = True

    def insert_act_table_loads(self):
        super().insert_act_table_loads()
        if not self.PIN_TABLES:
            return
        # Rewrite every emitted load to the one set that contains all our
        # functions, and keep only the first (straight-line kernel).
        from concourse.hw_specs import get_activation_tables

        names = list(get_activation_tables(self.m.arch).keys())
        target = names.index(_ONE_SET)
        first = True
        for fn in self.m.functions:
            for blk in fn.blocks:
                keep = []
                changed = False
                for inst in blk.instructions:
                    if type(inst).__name__ == "InstLoadActFuncSet":
                        if first:
                            inst.act_func_set_id = target
                            first = False
                            keep.append(inst)
                        else:
                            changed = True
                        continue
                    keep.append(inst)
                if changed:
                    del blk.instructions[:]
                    for i in keep:
                        blk.instructions.append(i)


def build_module(nj=L // LB, variant="full"):
    lt = nj * LB  # total l covered (full run: 4096)
    nc = _Bacc(
        "TRN2", target_bir_lowering=False, debug=False, num_devices=NCORES
    )
    q_d = nc.dram_tensor("query", [BLOC, H, 1, D], F32, kind="ExternalInput").ap()
    k_d = nc.dram_tensor("key", [BLOC, H, lt, D], F32, kind="ExternalInput").ap()
    m_d = nc.dram_tensor("mask", [BLOC, lt], I32, kind="ExternalInput").ap()
    o_d = nc.dram_tensor("out", [BLOC, H, lt], F32, kind="ExternalOutput").ap()

    with tile.TileContext(nc) as tc:
        with (
            tc.tile_pool(name="consts", bufs=1) as consts,
            tc.tile_pool(name="persist", bufs=1) as pers,
            tc.tile_pool(name="natp", bufs=16) as natp,
            tc.tile_pool(name="ktp", bufs=3) as ktp,
            tc.tile_pool(name="k2tp", bufs=3) as k2tp,
            tc.tile_pool(name="pst", bufs=3, space="PSUM") as pst,
            tc.tile_pool(name="psd", bufs=2, space="PSUM") as psd,
            tc.tile_pool(name="psn", bufs=2, space="PSUM") as psn,
            tc.tile_pool(name="psq", bufs=1, space="PSUM") as psq,
        ):
            # ---------------- prologue: constants -----------------
            ident = consts.tile([128, 128], F32)
            make_identity(nc, ident)
            identr = consts.tile([128, 128], F32R)
            nc.scalar.copy(identr[:], ident[:])

            qsb = pers.tile([NBH, D], F32, tag="qsb")
            nc.sync.dma_start(qsb[:], q_d.rearrange("b h o d -> (b h) (o d)"))

            # qn2[bh] = |q_bh|^2  (fused square+reduce on DVE)
            junkq = pers.tile([NBH, D], F32, tag="junkq")
            qn2 = pers.tile([NBH, 1], F32, tag="qn2")
            nc.vector.scalar_tensor_tensor(
                out=junkq[:],
                in0=qsb[:],
                scalar=1.0,
                in1=qsb[:],
                op0=mybir.AluOpType.mult,
                op1=mybir.AluOpType.mult,
                accum_out=qn2[:],
            )

            # qt [128(d), 32(bh)]
            qt_ps = psq.tile([128, NBH], F32, tag="qtps")
            nc.tensor.transpose(qt_ps[:], qsb[:], ident[0:NBH, 0:NBH])
            qt = pers.tile([128, NBH], F32, tag="qt")
            nc.scalar.copy(qt[:], qt_ps[:])

            # masked stationaries (fp32r, all ACT-produced):
            # MQ[:, bh, :] has q_bh in column bh, zeros elsewhere.
            # MONES[:, bh, :] has ones in column bh.
            mq = pers.tile([128, NBH, NBH], F32R, tag="mq")
            nc.scalar.activation(
                mq[:],
                qt[:].unsqueeze(1).broadcast_to([128, NBH, NBH]),
                AF.Copy,
                scale=0.0,
            )
            mones = pers.tile([128, NBH, NBH], F32R, tag="mones")
            nc.scalar.activation(
                mones[:],
                qt[:].unsqueeze(1).broadcast_to([128, NBH, NBH]),
                AF.Copy,
                scale=0.0,
            )
            for bh in range(NBH):
                nc.scalar.copy(mq[:, bh, bh : bh + 1], qt[:, bh : bh + 1])
                nc.scalar.activation(
                    mones[:, bh, bh : bh + 1],
                    qt[:, 0:1],
                    AF.Copy,
                    bias=1.0,
                    scale=0.0,
                )

            # mask as float, replicated over heads: row (b*16+h) = mask[b]
            maskf = pers.tile([NBH, lt], F32, tag="maskf")
            for bh in range(NBH):
                b = bh // H
                nc.gpsimd.dma_start(maskf[bh : bh + 1, :], m_d[b : b + 1, :])

            scores = pers.tile([NBH, lt], F32, tag="scores")
            kn2d = pers.tile([NBH, lt], F32, tag="kn2d")
            partials = pers.tile([NBH, nj], F32, tag="partials")

            # ---------------- main loop -----------------
            for j in range(nj):
                if variant == "full":
                    psd_t = psd.tile([NBH, LB], F32, tag="psd")
                    psn_t = psn.tile([NBH, LB], F32, tag="psn")
                for bh in range(NBH):
                    b, h = divmod(bh, H)
                    nat = natp.tile([128, NG, 128], F32R, tag="nat")
                    nc.sync.dma_start(
                        nat[:],
                        k_d[b, h, j * LB : (j + 1) * LB, :]
                        .rearrange("(g p) d -> p g d", p=128)
                        .bitcast(F32R),
                    )
                    if variant == "dmaonly":
                        continue
                    pt = pst.tile([128, LB], F32R, tag="pt")
                    for g in range(NG):
                        nc.tensor.matmul(
                            pt[:, g * 128 : (g + 1) * 128],
                            nat[:, g, :],
                            identr[:],
                            is_transpose=True,
                        )
                    if variant == "tponly":
                        continue
                    kt = ktp.tile([128, LB], F32R, tag="kt")
                    nc.vector.tensor_copy(kt[:], pt[:].bitcast(F32))
                    k2t = k2tp.tile([128, LB], F32R, tag="k2t")
                    nc.scalar.activation(k2t[:], pt[:].bitcast(F32), AF.Square)

                    if variant != "nomm":
                        nc.tensor.matmul(
                            psd_t[:],
                            mq[:, bh, :],
                            kt[:],
                            start=(bh == 0),
                            stop=(bh == NBH - 1),
                        )
                        nc.tensor.matmul(
                            psn_t[:],
                            mones[:, bh, :],
                            k2t[:],
                            start=(bh == 0),
                            stop=(bh == NBH - 1),
                        )

                sl = slice(j * LB, (j + 1) * LB)
                if variant in ("dmaonly", "tponly", "nomm"):
                    nc.vector.memset(scores[:, sl], 0.0)
                    nc.vector.memset(kn2d[:, sl], 1.0)
                else:
                    nc.vector.tensor_copy(scores[:, sl], psd_t[:])
                    nc.scalar.copy(kn2d[:, sl], psn_t[:])

                # per-j epilogue (all [32, 512] ops, overlapped with next j)
                nc.vector.tensor_scalar_mul(kn2d[:, sl], kn2d[:, sl], qn2[:])
                nc.scalar.activation(kn2d[:, sl], kn2d[:, sl], AF.Ln)
                nc.scalar.activation(kn2d[:, sl], kn2d[:, sl], AF.Exp, scale=-0.5)
                nc.vector.tensor_mul(scores[:, sl], scores[:, sl], kn2d[:, sl])
                nc.scalar.activation(scores[:, sl], scores[:, sl], AF.Exp)
                nc.vector.tensor_mul(scores[:, sl], scores[:, sl], maskf[:, sl])
                nc.vector.reduce_sum(
                    partials[:, j : j + 1], scores[:, sl], axis=AX.X
                )

            # ---------------- tail -----------------
            tot = pers.tile([NBH, 1], F32, tag="tot")
            nc.vector.reduce_sum(tot[:], partials[:], axis=AX.X)
            srec = pers.tile([NBH, 1], F32, tag="srec")
            nc.vector.reciprocal(srec[:], tot[:])
            nc.vector.tensor_scalar_mul(scores[:], scores[:], srec[:])
            nc.sync.dma_start(o_d.rearrange("b h l -> (b h) l"), scores[:])

    nc.compile()
    return nc


_CACHE = {}


def _get_module(nj=L // LB, variant="full"):
    key = (nj, variant)
    if key not in _CACHE:
        _CACHE[key] = build_module(nj, variant)
    return _CACHE[key]


def _run(query, key, mask, trace=False, nj=L // LB):
    nc = _get_module(nj)
    lt = nj * LB
    in_maps = []
    for c in range(NCORES):
        b0 = c * BLOC
        in_maps.append(
            {
                "query": np.ascontiguousarray(query[b0 : b0 + BLOC], np.float32),
                "key": np.ascontiguousarray(
                    key[b0 : b0 + BLOC, :, :lt], np.float32
                ),
                "mask": np.ascontiguousarray(mask[b0 : b0 + BLOC, :lt], np.int32),
            }
        )
    res = run_bass_kernel_spmd(
        nc, in_maps, core_ids=list(range(NCORES)), trace=trace
    )
    out = np.concatenate([r["out"] for r in res.results], axis=0)
    return out, res


def kernel(query, key, mask):
    out, _ = _run(np.asarray(query), np.asarray(key), np.asarray(mask))
    return out
